# revision 9
# baseline (speedup 1.0000x reference)
"""Trainium2 Bass kernel: batched 4-point DLT homography (closed-form solve).

Contract: kernel(pts_1_tile, pred_h4p_tile) -> [B, 3, 3] float32, with
B = 524288 split across 8 NeuronCores (batch-parallel, no communication).

Math (per batch element, points p=0..3 with src (x_p,y_p), dst (X_p,Y_p)):
the DLT system rows are
    x h0 + y h1 + h2 = X (1 + x h6 + y h7)
    x h3 + y h4 + h5 = Y (1 + x h6 + y h7)
Eliminating (h0,h1,h2) from the four X-equations via the left null vector n
of M = [(x_p, y_p, 1)] gives one linear equation in (h6,h7); same for the
Y-equations. Solve the 2x2, back out the rest in closed form.

Layout: each core's 65536 elements sit at [128 partitions, 512 free] split
into nchunk f-chunks; every per-element scalar is a [128, fc] fp16 "plane"
(DVE 2x mode). Engines execute their queues IN ORDER, so the chunks are
software-pipelined: each chunk body is a generator and the driver
interleaves emission across chunks with a skew, so one chunk's compute
fills the stalls of the other's cross-engine handoffs. The serial
dependency chain is pinned to VectorE; ScalarE takes the strided
deinterleave copies and output staging, GpSimd takes off-chain adds,
copies and a few mid-chain reduces.
"""
import sys

for _p in ("/opt/trn_rl_repo", "/root/.axon_site/_ro/trn_rl_repo"):
    if _p not in sys.path:
        sys.path.append(_p)

import numpy as np

import concourse.bass as bass
import concourse.mybir as mybir
from concourse import bacc
from concourse.tile import TileContext
from concourse.bass_utils import run_bass_kernel_spmd

N_CORES = 8
B_TOTAL = 524288
PER_CORE = B_TOTAL // N_CORES  # 65536
PARTS = 128
F = PER_CORE // PARTS  # 512
FP32 = mybir.dt.float32
FP16 = mybir.dt.float16

ADD = mybir.AluOpType.add
SUB = mybir.AluOpType.subtract
MUL = mybir.AluOpType.mult


class _Slab:
    """Bump allocator with explicit free, in F-plane units, first-fit."""

    def __init__(self, nplanes):
        self.free = [(0, nplanes)]
        self.nplanes = nplanes

    def alloc(self, n):
        for idx, (off, ln) in enumerate(self.free):
            if ln >= n:
                if ln == n:
                    self.free.pop(idx)
                else:
                    self.free[idx] = (off + n, ln - n)
                return off
        raise RuntimeError(f"slab OOM: need {n}, free={self.free}")

    def release(self, off, n):
        self.free.append((off, n))
        self.free.sort()
        merged = []
        for o, ln in self.free:
            if merged and merged[-1][0] + merged[-1][1] == o:
                merged[-1] = (merged[-1][0], merged[-1][1] + ln)
            else:
                merged.append([o, ln])
        self.free = [tuple(m) if isinstance(m, list) else m for m in merged]


def _fd(ap):
    n = 1
    for d in ap.shape[1:]:
        n *= d
    return n


OPLOG = {}


def _chunk_body(nc, pts, prd, out, pool32, poolp, c, fc, fp16):
    """Generator emitting one chunk's instructions; yields between ops so
    the driver can interleave chunks (software pipelining)."""
    hf = fc // 2
    elems = PARTS * fc
    PDT = FP16 if fp16 else FP32
    N32 = 30
    NP = 72

    slab32 = pool32.tile([PARTS, N32 * fc], FP32, tag="slab32")
    slabp = poolp.tile([PARTS, NP * fc], PDT, tag="slabp")
    sa32 = _Slab(N32)
    sa = _Slab(NP)

    def R32(off, n):
        return slab32[:, off * fc : (off + n) * fc]

    def R(off, n):
        return slabp[:, off * fc : (off + n) * fc]

    def V(off, n):
        return R(off, n).rearrange("p (c f) -> p c f", f=fc)

    def PL(off):
        return R(off, 1)

    def BC(off, k):
        return PL(off).unsqueeze(1).broadcast_to((PARTS, k, fc))

    ENG = {"v": nc.vector, "s": nc.scalar, "g": nc.gpsimd}

    def tt(o, a, b, op, eng="v", desc="tt"):
        ins = ENG[eng].tensor_tensor(out=o, in0=a, in1=b, op=op)
        OPLOG[ins.ins.name] = f"{desc}:{eng}"

    def cp(o, i, eng="s", desc="cp"):
        if eng == "s":
            ins = nc.scalar.copy(out=o, in_=i)
        elif eng == "v":
            ins = nc.vector.tensor_copy(out=o, in_=i)
        else:
            ins = nc.gpsimd.tensor_copy(out=o, in_=i)
        OPLOG[ins.ins.name] = f"{desc}:{eng}"

    def stt(o, in0, scalar, in1, op0, op1, desc="stt"):
        ins = nc.vector.scalar_tensor_tensor(
            out=o, in0=in0, scalar=scalar, in1=in1, op0=op0, op1=op1
        )
        OPLOG[ins.ins.name] = f"{desc}:v"

    lo = c * elems
    hi = lo + elems

    vt = sa32.alloc(8)
    pt = sa32.alloc(8)
    iv_all = pts[lo:hi, :].rearrange("(p f) c -> p (f c)", p=PARTS)
    ip_all = prd[lo:hi, :].rearrange("(p f) c -> p (f c)", p=PARTS)
    # f-halved input DMAs so deint can chase the first half
    nc.sync.dma_start(out=R32(vt, 8)[:, : 8 * hf], in_=iv_all[:, : 8 * hf])
    nc.sync.dma_start(out=R32(vt, 8)[:, 8 * hf :], in_=iv_all[:, 8 * hf :])
    nc.sync.dma_start(out=R32(pt, 8)[:, : 8 * hf], in_=ip_all[:, : 8 * hf])
    nc.sync.dma_start(out=R32(pt, 8)[:, 8 * hf :], in_=ip_all[:, 8 * hf :])
    yield

    # deinterleave (+ cast to fp16): (x0,y0,...,x3,y3) -> planar
    xv = sa.alloc(8)  # [x0,x1,x2,x3,y0,y1,y2,y3]
    pv = sa.alloc(8)  # pred planar, same order
    uu = sa.alloc(8)  # [X0,X1,X2,X3,Y0,Y1,Y2,Y3]

    iv = R32(vt, 8).rearrange("p (f c g) -> p g c f", c=4, g=2)
    ov_ = R(xv, 8).rearrange("p (g c f) -> p g c f", c=4, g=2)
    ipr = R32(pt, 8).rearrange("p (f c g) -> p g c f", c=4, g=2)
    op_ = R(pv, 8).rearrange("p (g c f) -> p g c f", c=4, g=2)
    # v-deint f-halved on ScalarE (x planes first so diffs start early)
    cp(ov_[:, 0, :, :hf], iv[:, 0, :, :hf], "s", desc="deint_vx1")
    yield
    cp(ov_[:, 0, :, hf:], iv[:, 0, :, hf:], "s", desc="deint_vx2")
    yield
    cp(ov_[:, 1, :, :hf], iv[:, 1, :, :hf], "s", desc="deint_vy1")
    yield
    cp(ov_[:, 1, :, hf:], iv[:, 1, :, hf:], "s", desc="deint_vy2")
    yield
    cp(op_[:, 0, :, :], ipr[:, 0, :, :], "s", desc="deint_px")
    yield
    cp(op_[:, 1, :, :], ipr[:, 1, :, :], "s", desc="deint_py")
    yield
    sa32.release(vt, 8)
    sa32.release(pt, 8)

    # u = v + pred, planar fp16, off-chain on GpSimd
    half = 4 * fc
    tt(R(uu, 8)[:, :half], R(xv, 8)[:, :half], R(pv, 8)[:, :half], ADD,
       "g", desc="uaddX")
    yield
    tt(R(uu, 8)[:, half:], R(xv, 8)[:, half:], R(pv, 8)[:, half:], ADD,
       "g", desc="uaddY")
    yield
    sa.release(pv, 8)

    ot = sa32.alloc(9)
    # OT is element-interleaved (f*9 + c): out-DMA is contiguous
    ov = R32(ot, 9).rearrange("p (f c) -> p c f", c=9)
    ins = nc.gpsimd.memset(ov[:, 8, :], 1.0)
    OPLOG[ins.ins.name] = "ones:g"
    yield

    # diffs: D = [dx1,dx2,dx3,dy1,dy2,dy3]
    dd = sa.alloc(6)
    xv3 = V(xv, 8)
    tt(V(dd, 6)[:, 0:3, :], xv3[:, 1:4, :], BC(xv, 3), SUB, "v", desc="diffx")
    yield
    tt(V(dd, 6)[:, 3:6, :], xv3[:, 5:8, :], BC(xv + 4, 3), SUB, "v",
       desc="diffy")
    yield
    DX1, DX2, DX3, DY1, DY2, DY3 = range(dd, dd + 6)

    # n: n1=dx2dy3-dx3dy2, n2=dx3dy1-dx1dy3, n3=dx1dy2-dx2dy1
    pa = sa.alloc(3)
    pb = sa.alloc(3)
    for k, (a, b) in enumerate(((DX2, DY3), (DX3, DY1), (DX1, DY2))):
        tt(PL(pa + k), PL(a), PL(b), MUL, "v", desc=f"pa{k}")
    yield
    for k, (a, b) in enumerate(((DX3, DY2), (DX1, DY3), (DX2, DY1))):
        tt(PL(pb + k), PL(a), PL(b), MUL, "v", desc=f"pb{k}")
    yield
    ns = sa.alloc(4)  # PDT [n0,n1,n2,n3]
    n3f = sa32.alloc(2)  # fp32 [n3, 1/n3]
    tt(V(ns, 4)[:, 1:3, :], R(pa, 2), R(pb, 2), SUB, "v", desc="n12")
    yield
    tt(R32(n3f, 1), PL(pa + 2), PL(pb + 2), SUB, "v", desc="n3_32")
    yield
    ins = nc.vector.reciprocal_approx_fast(out=R32(n3f + 1, 1), in_=R32(n3f, 1))
    OPLOG[ins.ins.name] = "rdV:v"
    yield
    rd = sa.alloc(1)
    cp(PL(ns + 3), R32(n3f, 1), "v", desc="n3cast")
    yield
    cp(PL(rd), R32(n3f + 1, 1), "v", desc="rdcast")
    yield
    t0 = sa.alloc(1)
    tt(PL(t0), PL(ns + 1), PL(ns + 2), ADD, "v", desc="t0")
    yield
    stt(PL(ns), PL(t0), -1.0, PL(ns + 3), MUL, SUB, desc="n0")  # n0=-(n1+n2)-n3
    yield
    sa32.release(n3f, 2)
    sa.release(pa, 3)
    sa.release(pb, 3)
    sa.release(t0, 1)

    # dots: for both sides W in {X,Y}: aW = sum n_p x_p W_p, bW = sum n_p
    # y_p W_p, cW = sum n_p W_p.  z8 = [zX0..3, zY0..3], zW_p = n_p W_p;
    # qr{X,Y} = [qW0..3, rW0..3] = zW*(x|y).  Pair-reduce then final.
    z8 = sa.alloc(8)
    tt(
        V(z8, 8).rearrange("p (s q) f -> p s q f", s=2),
        V(ns, 4).unsqueeze(1).broadcast_to((PARTS, 2, 4, fc)),
        V(uu, 8).rearrange("p (s q) f -> p s q f", s=2),
        MUL, "v", desc="z8",
    )
    yield
    qx = sa.alloc(8)
    qy = sa.alloc(8)
    xv4 = V(xv, 8).rearrange("p (a q) f -> p a q f", a=2)
    tt(V(qx, 8).rearrange("p (a q) f -> p a q f", a=2),
       V(z8, 8)[:, 0:4, :].unsqueeze(1).broadcast_to((PARTS, 2, 4, fc)),
       xv4, MUL, "v", desc="qrX")
    yield
    tt(V(qy, 8).rearrange("p (a q) f -> p a q f", a=2),
       V(z8, 8)[:, 4:8, :].unsqueeze(1).broadcast_to((PARTS, 2, 4, fc)),
       xv4, MUL, "v", desc="qrY")
    yield
    # pair reduce p:(0,1)+(2,3) -> r1 block [z(4), qrX(4), qrY(4)] then final
    r1 = sa.alloc(12)
    zv = V(z8, 8).rearrange("p (s t q) f -> p (s t) q f", t=1, q=4)
    tt(V(r1, 12)[:, 0:4, :].rearrange("p (s t) f -> p s t f", t=2),
       zv[:, :, 0:2, :], zv[:, :, 2:4, :], ADD, "g", desc="r1z")
    yield
    qxv = V(qx, 8).rearrange("p (s q) f -> p s q f", s=2)
    tt(V(r1, 12)[:, 4:8, :].rearrange("p (s t) f -> p s t f", t=2),
       qxv[:, :, 0:2, :], qxv[:, :, 2:4, :], ADD, "v", desc="r1x")
    yield
    qyv = V(qy, 8).rearrange("p (s q) f -> p s q f", s=2)
    tt(V(r1, 12)[:, 8:12, :].rearrange("p (s t) f -> p s t f", t=2),
       qyv[:, :, 0:2, :], qyv[:, :, 2:4, :], ADD, "g", desc="r1y")
    yield
    sa.release(z8, 8)
    sa.release(qx, 8)
    sa.release(qy, 8)
    # final reduce: ss6 = [cX,cY, aX,bX, aY,bY]
    ss = sa.alloc(6)
    r1v = V(r1, 12).rearrange("p (s t) f -> p s t f", t=2)
    tt(V(ss, 6), r1v[:, :, 0, :], r1v[:, :, 1, :], ADD, "v", desc="ss")
    yield
    sa.release(r1, 12)
    CX, CY, AX, BX, AY, BY = range(ss, ss + 6)

    # 2x2 system AX h6 + BX h7 + CX = 0; AY h6 + BY h7 + CY = 0:
    # det = AX BY - AY BX, h6n = BX CY - BY CX, h7n = AY CX - AX CY
    pc = sa.alloc(3)
    pd = sa.alloc(3)
    for k, (a, b) in enumerate(((AX, BY), (BX, CY), (AY, CX))):
        tt(PL(pc + k), PL(a), PL(b), MUL, "v", desc=f"pc{k}")
    yield
    for k, (a, b) in enumerate(((AY, BX), (BY, CX), (AX, CY))):
        tt(PL(pd + k), PL(a), PL(b), MUL, "v", desc=f"pd{k}")
    yield
    dtf = sa32.alloc(2)  # fp32 [det, 1/det]
    hn67 = sa.alloc(2)  # fp16 [h6n, h7n]
    tt(R32(dtf, 1), PL(pc), PL(pd), SUB, "v", desc="det32")
    yield
    tt(R(hn67, 2), R(pc + 1, 2), R(pd + 1, 2), SUB, "v", desc="hn67")
    yield
    ins = nc.vector.reciprocal_approx_fast(out=R32(dtf + 1, 1), in_=R32(dtf, 1))
    OPLOG[ins.ins.name] = "rdetV:v"
    yield
    rdet = sa.alloc(1)
    cp(PL(rdet), R32(dtf + 1, 1), "v", desc="rdetcast")
    yield
    sa.release(pc, 3)
    sa.release(pd, 3)
    sa.release(ss, 6)
    sa32.release(dtf, 2)

    h67 = sa.alloc(2)
    tt(V(h67, 2), V(hn67, 2), BC(rdet, 2), MUL, "v", desc="h67")
    yield
    cp(ov[:, 6:8, :], V(h67, 2), "s", desc="h67cp")
    yield
    sa.release(hn67, 2)
    sa.release(rdet, 1)

    # XW_p = X_p (1 + x_p h6 + y_p h7), p=0..2; same for YW.
    # sp = x h6 + y h7; XW/YW fused via scalar_tensor_tensor (sp+1)*U.
    m1 = sa.alloc(3)
    m2 = sa.alloc(3)
    sp = sa.alloc(3)
    xw = sa.alloc(6)  # [XW0,XW1,XW2,YW0,YW1,YW2]
    tt(V(m1, 3), V(xv, 8)[:, 0:3, :], BC(h67, 3), MUL, "v", desc="m1")
    yield
    tt(V(m2, 3), V(xv, 8)[:, 4:7, :], BC(h67 + 1, 3), MUL, "v", desc="m2")
    yield
    tt(R(sp, 3), R(m1, 3), R(m2, 3), ADD, "v", desc="sp")
    yield
    stt(V(xw, 6)[:, 0:3, :], R(sp, 3), 1.0, V(uu, 8)[:, 0:3, :], ADD, MUL,
        desc="XW")
    yield
    stt(V(xw, 6)[:, 3:6, :], R(sp, 3), 1.0, V(uu, 8)[:, 4:7, :], ADD, MUL,
        desc="YW")
    yield
    sa.release(m1, 3)
    sa.release(m2, 3)
    sa.release(sp, 3)
    sa.release(h67, 2)
    sa.release(uu, 8)

    # PQ = (P1, P2, Q1, Q2) = (XW1-XW0, XW2-XW0, YW1-YW0, YW2-YW0)
    pq = sa.alloc(4)
    xwv = R(xw, 6).rearrange("p (a b f) -> p a b f", a=2, b=3)
    tt(
        R(pq, 4).rearrange("p (a b f) -> p a b f", a=2, b=2),
        xwv[:, :, 1:3, :],
        xwv[:, :, 0, :].unsqueeze(2).broadcast_to((PARTS, 2, 2, fc)),
        SUB, "v", desc="PQ",
    )
    yield

    # pE = (P1 dy2, Q1 dy2, dx1 P2, dx1 Q2)
    # pF = (P2 dy1, Q2 dy1, dx2 P1, dx2 Q1)
    pe = sa.alloc(4)
    pf = sa.alloc(4)
    pqv = V(pq, 4)
    tt(V(pe, 4)[:, 0:2, :], pqv[:, 0:3:2, :], BC(DY2, 2), MUL, "v",
       desc="pe01")
    yield
    tt(V(pe, 4)[:, 2:4, :], pqv[:, 1:4:2, :], BC(DX1, 2), MUL, "v",
       desc="pe23")
    yield
    tt(V(pf, 4)[:, 0:2, :], pqv[:, 1:4:2, :], BC(DY1, 2), MUL, "v",
       desc="pf01")
    yield
    tt(V(pf, 4)[:, 2:4, :], pqv[:, 0:3:2, :], BC(DX2, 2), MUL, "v",
       desc="pf23")
    yield
    hn = sa.alloc(4)  # [h0n, h3n, h1n, h4n]
    tt(R(hn, 4), R(pe, 4), R(pf, 4), SUB, "g", desc="hn")
    yield
    hg = sa.alloc(4)  # [h0, h3, h1, h4]
    tt(V(hg, 4), V(hn, 4), BC(rd, 4), MUL, "v", desc="hg")
    yield
    sa.release(pe, 4)
    sa.release(pf, 4)
    sa.release(hn, 4)
    sa.release(pq, 4)
    sa.release(rd, 1)

    cp(ov[:, 0:4:3, :], V(hg, 2), "s", desc="hcopy03")
    yield
    cp(ov[:, 1:5:3, :], V(hg + 2, 2), "g", desc="hcopy14")
    yield

    # h2 = XW0 - x0 h0 - y0 h1 ; h5 = YW0 - x0 h3 - y0 h4
    # ee[t,s] = (x0,y0)[s] * hg[t,s]  (hg viewed [2,2] = [h0,h3; h1,h4])
    # ee = [x0h0, y0h1, x0h3, y0h4]: out[t,s] = (x0,y0)[s] * hg[(s t)]
    # (hg planes [h0,h3,h1,h4]; transposed view gives in1[t,s] = h(2s+t))
    ee = sa.alloc(4)
    xy0 = V(xv, 8)[:, 0:5:4, :]  # (x0, y0)
    tt(V(ee, 4).rearrange("p (t s) f -> p t s f", t=2),
       xy0.unsqueeze(1).broadcast_to((PARTS, 2, 2, fc)),
       V(hg, 4).rearrange("p (s t) f -> p t s f", s=2),
       MUL, "v", desc="ee")
    yield
    s1 = sa.alloc(2)
    eev = V(ee, 4)
    tt(V(s1, 2), V(xw, 6)[:, 0:4:3, :], eev[:, 0:3:2, :], SUB, "v", desc="s1")
    yield
    # h25 straight into fp32 output staging
    tt(ov[:, 2:6:3, :], V(s1, 2), eev[:, 1:4:2, :], SUB, "v", desc="h25")
    yield
    sa.release(ee, 4)
    sa.release(s1, 2)
    sa.release(hg, 4)
    sa.release(xw, 6)
    sa.release(dd, 6)
    sa.release(xv, 8)
    sa.release(ns, 4)

    nc.sync.dma_start(
        out=out[lo:hi, :].rearrange("(p f) c -> p (f c)", p=PARTS),
        in_=R32(ot, 9),
    )
    sa32.release(ot, 9)
    yield


def _build(nchunk=2, fp16=True, skew=14):
    OPLOG.clear()
    fc = F // nchunk

    nc = bacc.Bacc(None, target_bir_lowering=False, debug=True)
    pts = nc.dram_tensor("pts", [PER_CORE, 8], FP32, kind="ExternalInput")
    prd = nc.dram_tensor("prd", [PER_CORE, 8], FP32, kind="ExternalInput")
    out = nc.dram_tensor("out", [PER_CORE, 9], FP32, kind="ExternalOutput")

    with TileContext(nc) as tc:
        with tc.tile_pool(name="s32", bufs=2) as pool32, tc.tile_pool(
            name="sp", bufs=2
        ) as poolp:
            gens = [
                _chunk_body(nc, pts, prd, out, pool32, poolp, c, fc, fp16)
                for c in range(nchunk)
            ]
            active = []
            nxt = 0
            step = 0
            while active or nxt < len(gens):
                if nxt < len(gens) and step >= nxt * skew:
                    active.append(gens[nxt])
                    nxt += 1
                for g in list(active):
                    try:
                        next(g)
                    except StopIteration:
                        active.remove(g)
                step += 1
    nc.finalize()
    return nc


_NC_CACHE = {}


def _get_nc(nchunk=2, fp16=True):
    key = (nchunk, fp16)
    if key not in _NC_CACHE:
        _NC_CACHE[key] = _build(nchunk, fp16)
    return _NC_CACHE[key]


def kernel(pts_1_tile, pred_h4p_tile, _trace=False, _nchunk=2, _fp16=True):
    pts = np.ascontiguousarray(
        np.asarray(pts_1_tile, dtype=np.float32).reshape(B_TOTAL, 8)
    )
    prd = np.ascontiguousarray(
        np.asarray(pred_h4p_tile, dtype=np.float32).reshape(B_TOTAL, 8)
    )
    nc = _get_nc(_nchunk, _fp16)
    in_maps = [
        {
            "pts": pts[i * PER_CORE : (i + 1) * PER_CORE],
            "prd": prd[i * PER_CORE : (i + 1) * PER_CORE],
        }
        for i in range(N_CORES)
    ]
    res = run_bass_kernel_spmd(nc, in_maps, list(range(N_CORES)), trace=_trace)
    outs = np.concatenate([res.results[i]["out"] for i in range(N_CORES)], axis=0)
    H = outs.reshape(B_TOTAL, 3, 3).astype(np.float32)
    if _trace:
        return H, res
    return H
